# revision 1
# baseline (speedup 1.0000x reference)
"""Causal self-attention (B=8, T=1024, C=768, NH=12) on 8 TRN2 NeuronCores.

Sharding: pure data-parallel over batch — one batch element per core, weights
replicated. No collectives needed.

Per-core algorithm (all matmuls in float32r on the PE):
  1. xT = transpose(x)                        [C, T]   via PE transposes
  2. QT = (x @ Wq + bq)^T, KT = (x @ Wk + bk)^T        computed directly in
     transposed (channel-major) layout: QT_psum = Wq^T-tile.T... i.e.
     matmul(lhsT=W[:, ncols], rhs=xT) -> [n, m]; bias added via ACT eviction.
     V = x @ Wv + bv in natural layout: matmul(lhsT=xT, rhs=W).
  3. Per head h: ST[j, i] = KT_h[:, jblk].T @ QT_h  (keys on partitions).
     P = exp(ST * 1/sqrt(64)) via ACT (max-subtraction skipped: |S*scale| < 3),
     causal mask applied as a 0/1 multiply on the diagonal 128-block only.
     OT_aug = V_aug[jblk].T @ P accumulated over jblk in PSUM, where V_aug has
     64 ones-columns appended: PSUM rows 0:64 = unnormalized O^T, rows 64:128 =
     the softmax denominator broadcast 64x. Normalize with one reciprocal +
     multiply into OT (channel-major).
  4. y = OT.T @ Wp + bp  (OT is already the right lhsT layout).
"""
import numpy as np
from contextlib import ExitStack

import concourse.bass as bass
import concourse.tile as tile
from concourse import bacc, mybir
from concourse.bass_utils import run_bass_kernel_spmd
from concourse.masks import make_identity, make_upper_triangular

T, C, NH, HD = 1024, 768, 12, 64
N_CORES = 8
SCALE = 1.0 / 8.0  # 1/sqrt(HD)

F32 = mybir.dt.float32
F32R = mybir.dt.float32r
F16 = mybir.dt.float16
MM_DT = F32R  # matmul operand dtype (float32r: full-rate PE)
IDENT = mybir.ActivationFunctionType.Identity
EXP = mybir.ActivationFunctionType.Exp


def _body(ctx, tc, y, x, w_attn, b_attn, w_proj, b_proj):
    nc = tc.nc

    const = ctx.enter_context(tc.tile_pool(name="const", bufs=1))
    qk_pool = ctx.enter_context(tc.tile_pool(name="qk", bufs=1))
    v_pool = ctx.enter_context(tc.tile_pool(name="v", bufs=1))
    # single PSUM pool, 8 banks: tag "st" 2x[128,512] (transposes + ST tiles),
    # tag "mm" 2x[128,512] (QKV/proj groups), tag "ot" 4x[128,512] (O accum).
    psum = ctx.enter_context(tc.tile_pool(name="psum", bufs=2, space="PSUM"))

    # ---- constants ----
    ident = const.tile([128, 128], F32, tag="ident", name="ident")
    make_identity(nc, ident[:])
    # mask[j, i] = 1 if i >= j else 0  (keep key j for query i when i >= j)
    mask = const.tile([128, 128], F32, tag="mask", name="mask")
    make_upper_triangular(nc, mask[:], val=1.0, diag=True)

    # ---- persistent tensors ----
    QT = [qk_pool.tile([128, T], F16, tag=f"qt{i}", name=f"qt{i}") for i in range(6)]
    KT = [qk_pool.tile([128, T], F16, tag=f"kt{i}", name=f"kt{i}") for i in range(6)]
    # V_aug: 6 pair-groups of 192 cols: [V_{2p} (64) | ones (64) | V_{2p+1} (64)]
    # -> per-head lhsT is the contiguous 128-col slice [p*192 + (h%2)*64, +128):
    #    even head: [V_h | ones] (PSUM rows 0:64 = O^T, 64:128 = denom)
    #    odd head:  [ones | V_h] (rows flipped)
    V = [v_pool.tile([128, 1152], MM_DT, tag=f"v{i}", name=f"v{i}") for i in range(8)]
    for i in range(8):
        ones_ap = bass.AP(V[i].tensor, V[i].offset + 64, [V[i].ap[0], [192, 6], [1, 64]])
        nc.gpsimd.memset(ones_ap.bitcast(F32), 1.0)

    # ---- x load + transpose; weight loads; V projection ----
    xw_pool = ctx.enter_context(tc.tile_pool(name="xw", bufs=1))
    XT = [xw_pool.tile([128, T], MM_DT, tag=f"xt{i}", name=f"xt{i}") for i in range(6)]
    WQK = [xw_pool.tile([128, 2 * C], MM_DT, tag=f"w{i}", name=f"w{i}") for i in range(6)]

    with tc.tile_pool(name="xs", bufs=3) as x_pool, \
         tc.tile_pool(name="wv", bufs=1) as wv_pool:
        for mt in range(8):
            xm = x_pool.tile([128, C], F32, tag="x", name="xm")
            nc.sync.dma_start(xm[:], x[mt * 128:(mt + 1) * 128, :])
            for kc in range(6):
                tp = psum.tile([128, 512], F32, tag="st", name="tp")
                nc.tensor.transpose(tp[:, 0:128], xm[:, kc * 128:(kc + 1) * 128],
                                    ident[:])
                nc.vector.tensor_copy(XT[kc][:, mt * 128:(mt + 1) * 128],
                                      tp[:, 0:128])
        WV = [wv_pool.tile([128, C], MM_DT, tag=f"wv{i}", name=f"wv{i}")
              for i in range(6)]
        for k in range(6):
            nc.sync.dma_start(WV[k][:],
                              w_attn[k * 128:(k + 1) * 128, 2 * C:].bitcast(MM_DT))
        for k in range(6):
            nc.sync.dma_start(WQK[k][:],
                              w_attn[k * 128:(k + 1) * 128, 0:2 * C].bitcast(MM_DT))
        # biases: b_attn[0:1536] as [128, 12] (col t = b_attn[t*128:(t+1)*128])
        bqk = const.tile([128, 12], F32, tag="bqk", name="bqk")
        nc.sync.dma_start(bqk[:, :], b_attn[0:1536].rearrange("(n p) -> p n", p=128))
        bv_row = const.tile([1, C], F32, tag="bv_row", name="bv_row")
        nc.sync.dma_start(bv_row[:], b_attn[1536:2304].rearrange("(o f) -> o f", o=1))
        bv = const.tile([128, C], F32, tag="bv", name="bv")
        nc.gpsimd.partition_broadcast(bv[:], bv_row[:1, :])
        bp_row = const.tile([1, C], F32, tag="bp_row", name="bp_row")
        nc.sync.dma_start(bp_row[:], b_proj[:].rearrange("(o f) -> o f", o=1))
        bp = const.tile([128, C], F32, tag="bp", name="bp")
        nc.gpsimd.partition_broadcast(bp[:], bp_row[:1, :])

        for mt in range(8):
            for off, w in ((0, 512), (512, 256)):
                pv = psum.tile([128, 512], F32, tag="ot_ps", name="pv", bufs=4)
                for kc in range(6):
                    nc.tensor.matmul(
                        pv[:, :w], XT[kc][:, mt * 128:(mt + 1) * 128],
                        WV[kc][:, off:off + w],
                        start=(kc == 0), stop=(kc == 5))
                # scatter natural cols [off, off+w) into the pair-group layout,
                # one op per head parity
                a = w // 128
                p0 = off // 128
                for par in range(2):
                    src_ap = bass.AP(pv.tensor, pv.offset + par * 64,
                                     [pv.ap[0], [128, a], [1, 64]])
                    dst_ap = bass.AP(V[mt].tensor,
                                     V[mt].offset + p0 * 192 + par * 128,
                                     [V[mt].ap[0], [192, a], [1, 64]])
                    bv_ap = bass.AP(bv.tensor, bv.offset + off + par * 64,
                                    [bv.ap[0], [128, a], [1, 64]])
                    nc.vector.tensor_add(dst_ap, src_ap, bv_ap)

    # ---- QK projection (all pairs) ----
    for pr in range(6):
        for which, dst, boff in ((0, QT[pr], pr), (1, KT[pr], 6 + pr)):
            for mc in range(2):
                pq = psum.tile([128, 512], F32, tag="ot_ps", name="pq", bufs=4)
                for kc in range(6):
                    nc.tensor.matmul(
                        pq[:],
                        WQK[kc][:, which * C + pr * 128:which * C + (pr + 1) * 128],
                        XT[kc][:, mc * 512:(mc + 1) * 512],
                        start=(kc == 0), stop=(kc == 5))
                # bias-add + fp16 cast on DVE (keeps ACT free for exp)
                nc.vector.scalar_tensor_tensor(
                    dst[:, mc * 512:(mc + 1) * 512], pq[:],
                    bqk[:, boff:boff + 1], bv[:, 0:512],
                    op0=mybir.AluOpType.add, op1=mybir.AluOpType.bypass)

    # ---- attention, head pairs; single exp per (pair, jb, chunk) ----
    # ST for both heads of a pair lands in one 2-bank PSUM tile (even head at
    # [0:w], odd head at [512:512+w]) so ONE ACT exp covers both. pt_pair is
    # chunk-major: chunk c occupies cols [c*1024, c*1024+1024) with the even
    # head at +0 and the odd head at +512. Gap columns hold exp(garbage) and
    # are never read.
    ot_pool = ctx.enter_context(tc.tile_pool(name="ot", bufs=1))
    OT = [ot_pool.tile([128, T], MM_DT, tag=f"ot{i}", name=f"ot{i}") for i in range(6)]

    with tc.tile_pool(name="ptp", bufs=3) as pt_pool, \
         tc.tile_pool(name="nrm", bufs=4) as nrm_pool:
        for pr in range(6):
            QTt, KTt = QT[pr], KT[pr]
            # order: [h_even ci0, h_even ci1, h_odd ci0, h_odd ci1]
            ot_ps = [psum.tile([128, 512], F32, tag="ot_ps", name="ot_ps", bufs=4)
                     for _ in range(4)]
            for jb in range(8):
                jlo = jb * 128
                ptp = pt_pool.tile([128, 2048], MM_DT, tag="pt", name="ptp")
                for c in range((T - jlo + 511) // 512):
                    cs = jlo + c * 512
                    w = min(512, T - cs)
                    st = psum.tile([128, 1024], F32, tag="st", name="st")
                    for par in range(2):
                        nc.tensor.matmul(st[:, par * 512:par * 512 + w],
                                         KTt[par * 64:par * 64 + 64, jlo:jlo + 128],
                                         QTt[par * 64:par * 64 + 64, cs:cs + w],
                                         start=True, stop=True)
                    if w == 512:
                        nc.scalar.activation(ptp[:, c * 1024:c * 1024 + 1024],
                                             st[:, 0:1024], EXP, scale=SCALE)
                    else:
                        for par in range(2):
                            nc.scalar.activation(
                                ptp[:, c * 1024 + par * 512:c * 1024 + par * 512 + w],
                                st[:, par * 512:par * 512 + w], EXP, scale=SCALE)
                for par in range(2):
                    # causal mask on the diagonal 128-block (chunk 0, col 0)
                    diag = ptp[:, par * 512:par * 512 + 128]
                    nc.vector.tensor_mul(diag, diag, mask[:])
                    lhsT = V[jb][:, pr * 192 + par * 64:pr * 192 + par * 64 + 128]
                    for ci in range(2):
                        lo = ci * 512
                        if jlo >= lo + 512:
                            continue
                        s = max(jlo, lo)
                        e = lo + 512
                        # split at the ST chunk boundary jlo+512 if straddled
                        ranges = []
                        if s < jlo + 512:
                            ranges.append((s, min(e, jlo + 512), 0))
                        if e > jlo + 512:
                            ranges.append((max(s, jlo + 512), e, 1))
                        for (rs, re, c) in ranges:
                            rhs = ptp[:, c * 1024 + par * 512 + (rs - jlo - c * 512):
                                      c * 1024 + par * 512 + (rs - jlo - c * 512) + (re - rs)]
                            nc.tensor.matmul(
                                ot_ps[par * 2 + ci][:, rs - lo:re - lo],
                                lhsT, rhs,
                                start=(jb == 0), stop=(jb == 4 * ci + 3 and re == e))
            for par in range(2):
                o_rows = slice(0, 64) if par == 0 else slice(64, 128)
                d_rows = slice(64, 128) if par == 0 else slice(0, 64)
                for ci in range(2):
                    t = ot_ps[par * 2 + ci]
                    den = nrm_pool.tile([64, 512], F32, tag="den", name="den")
                    nc.vector.tensor_copy(den[:], t[d_rows, :])
                    recip = nrm_pool.tile([64, 512], F32, tag="recip", name="recip")
                    nc.vector.reciprocal_approx_fast(recip[:], den[:])
                    nc.vector.tensor_mul(
                        OT[pr][par * 64:(par + 1) * 64, ci * 512:(ci + 1) * 512],
                        t[o_rows, :], recip[:])

    # ---- output projection ----
    with tc.tile_pool(name="wp", bufs=1) as wp_pool, \
         tc.tile_pool(name="ysb", bufs=3) as y_pool:
        WP = [wp_pool.tile([128, C], MM_DT, tag=f"wp{i}", name=f"wp{i}")
              for i in range(6)]
        for k in range(6):
            nc.sync.dma_start(WP[k][:], w_proj[k * 128:(k + 1) * 128, :].bitcast(MM_DT))
        for mt in range(8):
            ysb = y_pool.tile([128, C], F32, tag="y", name="ysb")
            for off, w in ((0, 512), (512, 256)):
                py = psum.tile([128, 512], F32, tag="ot_ps", name="py", bufs=4)
                for kc in range(6):
                    nc.tensor.matmul(
                        py[:, :w], OT[kc][:, mt * 128:(mt + 1) * 128],
                        WP[kc][:, off:off + w],
                        start=(kc == 0), stop=(kc == 5))
                nc.vector.tensor_add(ysb[:, off:off + w], py[:, :w],
                                     bp[:, off:off + w])
            nc.sync.dma_start(y[mt * 128:(mt + 1) * 128, :], ysb[:])


_NC_CACHE = None


def _build():
    global _NC_CACHE
    if _NC_CACHE is not None:
        return _NC_CACHE
    nc = bacc.Bacc("TRN2", target_bir_lowering=False, debug=False,
                   num_devices=N_CORES)
    x = nc.dram_tensor("x", [T, C], F32, kind="ExternalInput").ap()
    w_attn = nc.dram_tensor("w_attn", [C, 3 * C], F32, kind="ExternalInput").ap()
    b_attn = nc.dram_tensor("b_attn", [3 * C], F32, kind="ExternalInput").ap()
    w_proj = nc.dram_tensor("w_proj", [C, C], F32, kind="ExternalInput").ap()
    b_proj = nc.dram_tensor("b_proj", [C], F32, kind="ExternalInput").ap()
    y = nc.dram_tensor("y", [T, C], F32, kind="ExternalOutput").ap()
    with tile.TileContext(nc) as tc, ExitStack() as ctx:
        _body(ctx, tc, y, x, w_attn, b_attn, w_proj, b_proj)
    nc.compile()
    _NC_CACHE = nc
    return nc


def _run(inputs, trace=False):
    nc = _build()
    x = np.ascontiguousarray(np.asarray(inputs["x"], dtype=np.float32))
    shared = {
        "w_attn": np.ascontiguousarray(np.asarray(inputs["w_attn"], np.float32)),
        "b_attn": np.ascontiguousarray(np.asarray(inputs["b_attn"], np.float32)),
        "w_proj": np.ascontiguousarray(np.asarray(inputs["w_proj"], np.float32)),
        "b_proj": np.ascontiguousarray(np.asarray(inputs["b_proj"], np.float32)),
    }
    in_maps = [dict(x=np.ascontiguousarray(x[b]), **shared) for b in range(N_CORES)]
    res = run_bass_kernel_spmd(nc, in_maps, core_ids=list(range(N_CORES)),
                               trace=trace)
    out = np.stack([res.results[b]["y"] for b in range(N_CORES)], axis=0)
    return out.astype(np.float32), res


def kernel(**inputs):
    out, _ = _run(inputs, trace=False)
    return out



# revision 2
# speedup vs baseline: 1.1091x; 1.1091x over previous
"""Causal self-attention (B=8, T=1024, C=768, NH=12) on 8 TRN2 NeuronCores.

Sharding: pure data-parallel over batch — one batch element per core, weights
replicated. No collectives.

v2 vs baseline (229us): all matmul operands fp16 (host-cast: halves DMA bytes,
enables fast-weight-load, same 1 cyc/row PE rate), K-bias dropped (softmax
shift-invariance), V-bias folded into a host-precomputed output bias, and the
whole program is software-pipelined so the PE never idles (QK projection of
pair p+2 is interleaved as PE filler under pair p's softmax exp, PV matmuls lag
one slot behind their exp). PSUM: st 2x[128,1024] (QK^T), ot 1x[128,1024]
(PV accum per (pair, query-chunk)), mm 2x1-bank (all projection groups).

Per-core algorithm:
  1. xT tiles via PE transposes (fp16, identity moving).
  2. V = x @ Wv (no bias) scattered into pair-group layout with 64 ones-cols
     per head pair half: per 192-col group [V_even | ones | V_odd].
  3. QT/KT per head pair: matmul(lhsT=Wqk col-block, rhs=xT) -> channel-major;
     Q bias added on DVE eviction (STT), K bias dropped (cancels in softmax).
  4. Attention per pair, query-chunk ci (512), key-block jb (128):
     ST = KT_h[jb].T @ QT_h (two heads row-tiled concurrently), exp on ACT
     (scale 1/8, no max-sub: |S/8|<4), causal mask 0/1 multiply on the
     diagonal block only, OT_aug += V_aug[jb].T @ P accumulated in PSUM
     (rows: 64 O^T + 64 denominator). Normalize with reciprocal + mul.
  5. y = OT.T @ Wp + (bv @ Wp + bp)  [second term host-precomputed].
"""
import numpy as np
from contextlib import ExitStack

import concourse.bass as bass
import concourse.tile as tile
from concourse import bacc, mybir
from concourse.bass_utils import run_bass_kernel_spmd
from concourse.masks import make_identity, make_upper_triangular

T, C, NH, HD = 1024, 768, 12, 64
N_CORES = 8
SCALE = 1.0 / 8.0  # 1/sqrt(HD)

F32 = mybir.dt.float32
F16 = mybir.dt.float16
EXP = mybir.ActivationFunctionType.Exp


def _body(ctx, tc, y, x, w_attn, bq_d, bp_d, w_proj):
    nc = tc.nc

    const = ctx.enter_context(tc.tile_pool(name="const", bufs=1))
    persist = ctx.enter_context(tc.tile_pool(name="persist", bufs=1))
    # PSUM: st 2x[128,1024]f32 (4 banks), ot 1x[128,1024]f32 (2 banks),
    # mm 2x 1-bank (projection groups / transposes).
    psum = ctx.enter_context(tc.tile_pool(name="psum", bufs=1, space="PSUM"))

    # ---- constants ----
    ident = const.tile([128, 128], F16, tag="ident", name="ident")
    make_identity(nc, ident[:])
    mask = const.tile([128, 128], F16, tag="mask", name="mask")
    make_upper_triangular(nc, mask[:], val=1.0, diag=True)
    bq = const.tile([128, 6], F32, tag="bq", name="bq")
    nc.sync.dma_start(bq[:, :], bq_d[:].rearrange("(n p) -> p n", p=128))
    bp_row = const.tile([1, C], F32, tag="bp_row", name="bp_row")
    nc.sync.dma_start(bp_row[:], bp_d[:].rearrange("(o f) -> o f", o=1))
    bp = const.tile([128, C], F32, tag="bp", name="bp")
    nc.gpsimd.partition_broadcast(bp[:], bp_row[:1, :])

    # ---- persistent tensors ----
    # XT: single tile, kc-major: [128, kc*1024 + t]
    XT = persist.tile([128, 6 * T], F16, tag="xt", name="xt")
    WQK = [persist.tile([128, 2 * C], F16, tag=f"wqk{i}", name=f"wqk{i}")
           for i in range(6)]
    WV = [persist.tile([128, C], F16, tag=f"wv{i}", name=f"wv{i}")
          for i in range(6)]
    WP = [persist.tile([128, C], F16, tag=f"wp{i}", name=f"wp{i}")
          for i in range(6)]
    QT = [persist.tile([128, T], F16, tag=f"qt{i}", name=f"qt{i}") for i in range(6)]
    KT = [persist.tile([128, T], F16, tag=f"kt{i}", name=f"kt{i}") for i in range(6)]
    OT = [persist.tile([128, T], F16, tag=f"ot{i}", name=f"ot{i}") for i in range(6)]
    # V_aug: 6 pair-groups of 192 cols: [V_{2p} (64) | ones (64) | V_{2p+1} (64)]
    V = [persist.tile([128, 1152], F16, tag=f"v{i}", name=f"v{i}") for i in range(8)]
    for i in range(8):
        ones_ap = bass.AP(V[i].tensor, V[i].offset + 64,
                          [V[i].ap[0], [192, 6], [1, 64]])
        nc.gpsimd.memset(ones_ap.bitcast(F16), 1.0)

    # ---- input DMAs (issued up front; deps tracked by Tile) ----
    x_pool = ctx.enter_context(tc.tile_pool(name="xs", bufs=3))
    XM = []
    for mt in range(8):
        xm = x_pool.tile([128, C], F16, tag="x", name=f"xm{mt}")
        nc.sync.dma_start(xm[:], x[mt * 128:(mt + 1) * 128, :])
        XM.append(xm)
    for k in range(6):
        nc.sync.dma_start(WV[k][:], w_attn[k * 128:(k + 1) * 128, 2 * C:])
    for k in range(6):
        nc.sync.dma_start(WQK[k][:], w_attn[k * 128:(k + 1) * 128, 0:2 * C])
    for k in range(6):
        nc.sync.dma_start(WP[k][:], w_proj[k * 128:(k + 1) * 128, :])

    # ---- phase 1: transposes + V projection, per token block ----
    for mt in range(8):
        tp = psum.tile([128, C], F16, tag="mm", name="tp", bufs=2)
        for kc in range(6):
            nc.tensor.transpose(tp[:, kc * 128:(kc + 1) * 128],
                                XM[mt][:, kc * 128:(kc + 1) * 128], ident[:])
        # one strided eviction: psum col kc*128+i -> XT col kc*1024 + mt*128 + i
        dst = bass.AP(XT.tensor, XT.offset + mt * 128,
                      [XT.ap[0], [T, 6], [1, 128]])
        nc.vector.tensor_copy(dst, tp[:, :])
        for off, w in ((0, 512), (512, 256)):
            pv = psum.tile([128, 512], F32, tag="mm", name="pv", bufs=2)
            for kc in range(6):
                nc.tensor.matmul(
                    pv[:, :w], XT[:, kc * T + mt * 128:kc * T + (mt + 1) * 128],
                    WV[kc][:, off:off + w], start=(kc == 0), stop=(kc == 5))
            # scatter natural cols [off, off+w) into pair-group layout
            a = w // 128
            p0 = off // 128
            for par in range(2):
                src_ap = bass.AP(pv.tensor, pv.offset + par * 64,
                                 [pv.ap[0], [128, a], [1, 64]])
                dst_ap = bass.AP(V[mt].tensor,
                                 V[mt].offset + p0 * 192 + par * 128,
                                 [V[mt].ap[0], [192, a], [1, 64]])
                nc.vector.tensor_copy(dst_ap.bitcast(F16), src_ap)

    # ---- QK projection group emitter (4 groups of 6 matmuls per pair) ----
    def qk_group_ops(pr):
        """Yield closures: 6 matmul-emitters then 1 eviction-emitter, x4."""
        for which in range(2):  # 0 = Q, 1 = K
            for mc in range(2):
                pq = psum.tile([128, 512], F32, tag="mm",
                               name=f"pq{pr}{which}{mc}", bufs=2)

                def mm(kc, pq=pq, which=which, pr=pr, mc=mc):
                    nc.tensor.matmul(
                        pq[:],
                        WQK[kc][:, which * C + pr * 128:which * C + (pr + 1) * 128],
                        XT[:, kc * T + mc * 512:kc * T + (mc + 1) * 512],
                        start=(kc == 0), stop=(kc == 5))
                for kc in range(6):
                    yield lambda kc=kc, mm=mm: mm(kc)

                def ev(pq=pq, which=which, pr=pr, mc=mc):
                    dst = (QT if which == 0 else KT)[pr][:, mc * 512:(mc + 1) * 512]
                    if which == 0:
                        nc.vector.scalar_tensor_tensor(
                            dst, pq[:], bq[:, pr:pr + 1], XT[:, 0:512],
                            op0=mybir.AluOpType.add, op1=mybir.AluOpType.bypass)
                    else:
                        nc.vector.tensor_copy(dst, pq[:])
                yield ev

    # ---- phase 2: QK projection for pairs 0 and 1 ----
    for pr in (0, 1):
        for op in qk_group_ops(pr):
            op()

    # ---- phase 3: attention, pair p with pair p+2's projection as filler ----
    pt_pool = ctx.enter_context(tc.tile_pool(name="ptp", bufs=3))
    nrm_pool = ctx.enter_context(tc.tile_pool(name="nrm", bufs=4))

    def norm(pr, ci, ot):
        for par in range(2):
            o_rows = slice(0, 64) if par == 0 else slice(64, 128)
            d_rows = slice(64, 128) if par == 0 else slice(0, 64)
            c0 = par * 512
            den = nrm_pool.tile([64, 512], F32, tag="den", name="den")
            nc.vector.tensor_copy(den[:], ot[d_rows, c0:c0 + 512])
            recip = nrm_pool.tile([64, 512], F32, tag="recip", name="recip")
            nc.vector.reciprocal_approx_fast(recip[:], den[:])
            nc.vector.tensor_mul(
                OT[pr][par * 64:(par + 1) * 64, ci * 512:(ci + 1) * 512],
                ot[o_rows, c0:c0 + 512], recip[:])

    for pr in range(6):
        filler = list(qk_group_ops(pr + 2)) if pr < 4 else []
        fi = 0
        pend = None  # (pv_emitter, norm_emitter_or_None)
        for ci in range(2):
            ot = psum.tile([128, 1024], F32, tag="ot", name=f"ot{pr}{ci}")
            njb = 4 * ci + 4
            for jb in range(njb):
                jlo = jb * 128
                lo = max(0, jlo - ci * 512)  # first valid col in this chunk
                w = 512 - lo
                st = psum.tile([128, 1024], F32, tag="st", name="st", bufs=2)
                for par in range(2):
                    nc.tensor.matmul(
                        st[:, par * 512 + lo:par * 512 + lo + w],
                        KT[pr][par * 64:par * 64 + 64, jlo:jlo + 128],
                        QT[pr][par * 64:par * 64 + 64,
                               ci * 512 + lo:ci * 512 + lo + w],
                        start=True, stop=True)
                ptp = pt_pool.tile([128, 1024], F16, tag="pt", name="ptp")
                if w == 512:
                    nc.scalar.activation(ptp[:, :], st[:, :], EXP, scale=SCALE)
                else:
                    for par in range(2):
                        nc.scalar.activation(
                            ptp[:, par * 512 + lo:par * 512 + lo + w],
                            st[:, par * 512 + lo:par * 512 + lo + w],
                            EXP, scale=SCALE)
                if jb >= 4 * ci:  # diagonal block lives in this chunk
                    for par in range(2):
                        diag = ptp[:, par * 512 + lo:par * 512 + lo + 128]
                        nc.vector.tensor_mul(diag, diag, mask[:])
                # PE filler: 2 projection ops per slot
                for _ in range(2):
                    if fi < len(filler):
                        filler[fi]()
                        fi += 1
                # previous slot's PV (+ pending norm)
                if pend is not None:
                    pend[0]()
                    if pend[1] is not None:
                        pend[1]()

                def pv(pr=pr, ci=ci, jb=jb, njb=njb, lo=lo, w=w, ot=ot, ptp=ptp):
                    for par in range(2):
                        nc.tensor.matmul(
                            ot[:, par * 512 + lo:par * 512 + lo + w],
                            V[jb][:, pr * 192 + par * 64:pr * 192 + par * 64 + 128],
                            ptp[:, par * 512 + lo:par * 512 + lo + w],
                            start=(jb == 0), stop=(jb == njb - 1))
                last = (jb == njb - 1)
                pend = (pv, (lambda pr=pr, ci=ci, ot=ot: norm(pr, ci, ot))
                        if last else None)
        # flush at pair end
        pend[0]()
        pend[1]()
        pend = None
        while fi < len(filler):
            filler[fi]()
            fi += 1

    # ---- phase 4: output projection ----
    y_pool = ctx.enter_context(tc.tile_pool(name="ysb", bufs=3))
    for mt in range(8):
        ysb = y_pool.tile([128, C], F32, tag="y", name="ysb")
        for off, w in ((0, 512), (512, 256)):
            py = psum.tile([128, 512], F32, tag="mm", name="py", bufs=2)
            for kc in range(6):
                nc.tensor.matmul(
                    py[:, :w], OT[kc][:, mt * 128:(mt + 1) * 128],
                    WP[kc][:, off:off + w], start=(kc == 0), stop=(kc == 5))
            nc.vector.tensor_add(ysb[:, off:off + w], py[:, :w],
                                 bp[:, off:off + w])
        nc.sync.dma_start(y[mt * 128:(mt + 1) * 128, :], ysb[:])


_NC_CACHE = None


def _build():
    global _NC_CACHE
    if _NC_CACHE is not None:
        return _NC_CACHE
    nc = bacc.Bacc("TRN2", target_bir_lowering=False, debug=False,
                   num_devices=N_CORES)
    x = nc.dram_tensor("x", [T, C], F16, kind="ExternalInput").ap()
    w_attn = nc.dram_tensor("w_attn", [C, 3 * C], F16, kind="ExternalInput").ap()
    bq_d = nc.dram_tensor("bq", [C], F32, kind="ExternalInput").ap()
    bp_d = nc.dram_tensor("bp_eff", [C], F32, kind="ExternalInput").ap()
    w_proj = nc.dram_tensor("w_proj", [C, C], F16, kind="ExternalInput").ap()
    y = nc.dram_tensor("y", [T, C], F32, kind="ExternalOutput").ap()
    with tile.TileContext(nc) as tc, ExitStack() as ctx:
        _body(ctx, tc, y, x, w_attn, bq_d, bp_d, w_proj)
    nc.compile()
    _NC_CACHE = nc
    return nc


def _run(inputs, trace=False):
    nc = _build()
    x = np.asarray(inputs["x"], dtype=np.float32)
    b_attn = np.asarray(inputs["b_attn"], dtype=np.float64)
    w_proj = np.asarray(inputs["w_proj"], dtype=np.float64)
    b_proj = np.asarray(inputs["b_proj"], dtype=np.float64)
    # K bias dropped (cancels in softmax); V bias folded into output bias:
    # y = O@Wp + (bv@Wp + bp)
    bp_eff = (b_attn[2 * C:] @ w_proj + b_proj).astype(np.float32)
    shared = {
        "w_attn": np.ascontiguousarray(
            np.asarray(inputs["w_attn"], np.float32).astype(np.float16)),
        "bq": np.ascontiguousarray(b_attn[0:C].astype(np.float32)),
        "bp_eff": np.ascontiguousarray(bp_eff),
        "w_proj": np.ascontiguousarray(
            np.asarray(inputs["w_proj"], np.float32).astype(np.float16)),
    }
    x16 = x.astype(np.float16)
    in_maps = [dict(x=np.ascontiguousarray(x16[b]), **shared)
               for b in range(N_CORES)]
    res = run_bass_kernel_spmd(nc, in_maps, core_ids=list(range(N_CORES)),
                               trace=trace)
    out = np.stack([res.results[b]["y"] for b in range(N_CORES)], axis=0)
    return out.astype(np.float32), res


def kernel(**inputs):
    out, _ = _run(inputs, trace=False)
    return out


# revision 13
# speedup vs baseline: 1.1252x; 1.0146x over previous
"""Causal self-attention (B=8, T=1024, C=768, NH=12) on 8 TRN2 NeuronCores.

Sharding: pure data-parallel over batch — one batch element per core, weights
replicated. No collectives.

All matmul operands fp16 (host-cast: halves DMA, enables fast-weight-load,
1 cyc/row PE rate). K-bias dropped (cancels in softmax); V-bias folded into a
host-precomputed output bias. The program is software-pipelined so the PE
never idles: QK projection of pair p+2 runs as PE filler under pair p's
softmax exp, PV matmuls lag one slot behind their exp, PV accumulators are
released by one fast fp16 copy (normalization happens off-path on DVE).

Per-core algorithm:
  1. xT tiles via PE transposes (fp16 identity moving operand).
  2. V = x @ Wv scattered into 256-col pair-groups [V_even|ones|V_odd|ones]
     so both heads' PV lhsT is [V_h(64) | ones(64)] -> PSUM rows 0:64 = O^T,
     64:128 = softmax denominator.
  3. QT/KT channel-major via matmul(lhsT=Wqk col-block, rhs=xT); Q bias via
     DVE STT eviction; K bias dropped.
  4. Attention per (pair, query-chunk ci of 512, key-block jb of 128):
     ST = KT_h[jb].T @ QT_h (heads row-tiled concurrent), exp on ACT
     (scale 1/8, no max-sub), causal 0/1 mask on the diagonal block only,
     OT_aug += V_aug[jb].T @ P in PSUM. Unnormalized copy to SBUF (fp16),
     then reciprocal + 2 muls produce OT.
  5. y = OT.T @ Wp + (bv @ Wp + bp).
"""
import numpy as np
from contextlib import ExitStack

import concourse.bass as bass
import concourse.tile as tile
from concourse import bacc, mybir
from concourse.bass_utils import run_bass_kernel_spmd
from concourse.masks import make_identity, make_upper_triangular

T, C, NH, HD = 1024, 768, 12, 64
N_CORES = 8
SCALE = 1.0 / 8.0  # 1/sqrt(HD)

F32 = mybir.dt.float32
F16 = mybir.dt.float16
EXP = mybir.ActivationFunctionType.Exp


def _body(ctx, tc, y, x, w_attn, bq_d, bp_d, w_proj):
    nc = tc.nc

    const = ctx.enter_context(tc.tile_pool(name="const", bufs=1))
    persist = ctx.enter_context(tc.tile_pool(name="persist", bufs=1))
    # PSUM: st 2x[128,1024]f32 (4 banks), ot 1x[128,1024]f32 (2 banks),
    # mm 2x 1-bank (transposes / projection groups).
    psum = ctx.enter_context(tc.tile_pool(name="psum", bufs=1, space="PSUM"))

    # ---- constants ----
    ident = const.tile([128, 128], F16, tag="ident", name="ident")
    make_identity(nc, ident[:])
    bq = const.tile([128, 6], F32, tag="bq", name="bq")
    nc.sync.dma_start(bq[:, :], bq_d[:].rearrange("(n p) -> p n", p=128))
    bp_row = const.tile([1, C], F32, tag="bp_row", name="bp_row")
    nc.sync.dma_start(bp_row[:], bp_d[:].rearrange("(o f) -> o f", o=1))
    bp = const.tile([128, C], F32, tag="bp", name="bp")
    nc.gpsimd.partition_broadcast(bp[:], bp_row[:1, :])

    # ---- persistent tensors ----
    XT = persist.tile([128, 6 * T], F16, tag="xt", name="xt")  # kc-major
    WQK = [persist.tile([128, 2 * C], F16, tag=f"wqk{i}", name=f"wqk{i}")
           for i in range(6)]
    WV = [persist.tile([128, C], F16, tag=f"wv{i}", name=f"wv{i}")
          for i in range(6)]
    WP = [persist.tile([128, C], F16, tag=f"wp{i}", name=f"wp{i}")
          for i in range(6)]
    QT = [persist.tile([128, T], F16, tag=f"qt{i}", name=f"qt{i}") for i in range(6)]
    KT = [persist.tile([128, T], F16, tag=f"kt{i}", name=f"kt{i}") for i in range(6)]
    OT = [persist.tile([128, T], F16, tag=f"ot{i}", name=f"ot{i}") for i in range(6)]
    # V_aug: 6 pair-groups of 192 cols: [V_{2p}(64) | ones(64) | V_{2p+1}(64)]
    # -> even head lhsT [V|ones] (PSUM rows 0:64 = O^T, 64:128 = denom);
    #    odd head lhsT [ones|V] (rows flipped)
    V = [persist.tile([128, 1152], F16, tag=f"v{i}", name=f"v{i}") for i in range(8)]
    for i in range(8):
        ones_ap = bass.AP(V[i].tensor, V[i].offset + 64,
                          [V[i].ap[0], [192, 6], [1, 64]])
        nc.gpsimd.memset(ones_ap.bitcast(F16), 1.0)

    # ---- input DMAs: x/y on sync queue, weights on gpsimd queue ----
    x_pool = ctx.enter_context(tc.tile_pool(name="xs", bufs=1))
    XM = [x_pool.tile([128, C], F16, tag=f"x{mt}", name=f"xm{mt}")
          for mt in range(8)]
    for mt in range(4):
        nc.sync.dma_start(XM[mt][:], x[mt * 128:(mt + 1) * 128, :])
    for k in range(6):
        nc.gpsimd.dma_start(WV[k][:], w_attn[k * 128:(k + 1) * 128, 2 * C:])
    for mt in range(4, 8):
        nc.sync.dma_start(XM[mt][:], x[mt * 128:(mt + 1) * 128, :])
    for k in range(6):
        nc.gpsimd.dma_start(WQK[k][:], w_attn[k * 128:(k + 1) * 128, 0:2 * C])
    for k in range(6):
        nc.gpsimd.dma_start(WP[k][:], w_proj[k * 128:(k + 1) * 128, :])

    # ---- PE warmup: junk transposes to lift the HAM clock gate early ----
    warm = psum.tile([128, 512], F32, tag="mm", name="warm", bufs=2)
    for i in range(32):
        nc.tensor.transpose(warm.bitcast(F16)[:, (i % 8) * 128:(i % 8) * 128 + 128],
                            ident[:], ident[:])

    # ---- phase 1: transposes + V projection, per token block ----
    def phase1_units(mt):
        def transp(mt=mt):
            tp = psum.tile([128, C], F16, tag="mm", name="tp", bufs=2)
            for kc in range(6):
                nc.tensor.transpose(tp[:, kc * 128:(kc + 1) * 128],
                                    XM[mt][:, kc * 128:(kc + 1) * 128], ident[:])
            dst = bass.AP(XT.tensor, XT.offset + mt * 128,
                          [XT.ap[0], [T, 6], [1, 128]])
            nc.vector.tensor_copy(dst, tp[:, :])
        yield transp
        for off, w in ((0, 512), (512, 256)):
            def vproj(mt=mt, off=off, w=w):
                pv = psum.tile([128, 512], F32, tag="mm", name="pv", bufs=2)
                for kc in range(6):
                    nc.tensor.matmul(
                        pv[:, :w],
                        XT[:, kc * T + mt * 128:kc * T + (mt + 1) * 128],
                        WV[kc][:, off:off + w], start=(kc == 0), stop=(kc == 5))
                a = w // 128
                p0 = off // 128
                for par in range(2):
                    src_ap = bass.AP(pv.tensor, pv.offset + par * 64,
                                     [pv.ap[0], [128, a], [1, 64]])
                    dst_ap = bass.AP(V[mt].tensor,
                                     V[mt].offset + p0 * 192 + par * 128,
                                     [V[mt].ap[0], [192, a], [1, 64]])
                    nc.vector.tensor_copy(dst_ap.bitcast(F16), src_ap)
            yield vproj

    for mt in range(8):
        for op in phase1_units(mt):
            op()

    # ---- QK projection group emitter (4 groups of 6 matmuls per pair) ----
    def qk_group_ops(pr):
        for which in range(2):  # 0 = Q, 1 = K
            for mc in range(2):
                pq = psum.tile([128, 512], F32, tag="mm",
                               name=f"pq{pr}{which}{mc}", bufs=2)

                def mm(kc, pq=pq, which=which, pr=pr, mc=mc):
                    nc.tensor.matmul(
                        pq[:],
                        WQK[kc][:, which * C + pr * 128:which * C + (pr + 1) * 128],
                        XT[:, kc * T + mc * 512:kc * T + (mc + 1) * 512],
                        start=(kc == 0), stop=(kc == 5))
                for kc in range(6):
                    yield lambda kc=kc, mm=mm: mm(kc)

                def ev(pq=pq, which=which, pr=pr, mc=mc):
                    dst = (QT if which == 0 else KT)[pr][:, mc * 512:(mc + 1) * 512]
                    if which == 0:
                        nc.vector.scalar_tensor_tensor(
                            dst, pq[:], bq[:, pr:pr + 1], XT[:, 0:512],
                            op0=mybir.AluOpType.add, op1=mybir.AluOpType.bypass)
                    else:
                        nc.vector.tensor_copy(dst, pq[:])
                yield ev

    # ---- phase 2: QK projection for pairs 0 and 1 ----
    for pr in (0, 1):
        for op in qk_group_ops(pr):
            op()

    # ---- phase 3: attention; pair p runs pair p+2's projection as filler ----
    pt_pool = ctx.enter_context(tc.tile_pool(name="ptp", bufs=3))
    nrm_pool = ctx.enter_context(tc.tile_pool(name="nrm", bufs=2))

    def norm(pr, ci, ot):
        # par0 (cols 0:512): O rows 0:64, den rows 64:128;
        # par1 (cols 512:1024): den rows 0:64, O rows 64:128.
        den = nrm_pool.tile([64, 1024], F32, tag="den", name="den")
        nc.vector.tensor_copy(den[:, 0:512], ot[64:128, 0:512])
        nc.vector.tensor_copy(den[:, 512:1024], ot[0:64, 512:1024])
        recip = nrm_pool.tile([64, 1024], F32, tag="recip", name="recip")
        nc.vector.reciprocal_approx_fast(recip[:], den[:])
        nc.vector.tensor_mul(OT[pr][0:64, ci * 512:(ci + 1) * 512],
                             ot[0:64, 0:512], recip[0:64, 0:512])
        nc.vector.tensor_mul(OT[pr][64:128, ci * 512:(ci + 1) * 512],
                             ot[64:128, 512:1024], recip[0:64, 512:1024])

    # filler matmuls per slot: front-load chunk starts so the PE has work
    # while the previous chunk's accumulator is drained by norm()
    FILL = {0: [3, 3, 1, 1], 1: [3, 3, 2, 2, 2, 2, 1, 1]}
    for pr in range(6):
        filler = list(qk_group_ops(pr + 2)) if pr < 4 else []
        fi = 0
        pend = None  # (pv_emitter, norm_emitter_or_None)
        for ci in range(2):
            ot = psum.tile([128, 1024], F32, tag="ot", name=f"ot{pr}{ci}")
            njb = 4 * ci + 4
            for jb in range(njb):
                jlo = jb * 128
                lo = max(0, jlo - ci * 512)  # first valid col in this chunk
                w = 512 - lo
                st = psum.tile([128, 1024], F32, tag="st", name="st", bufs=2)
                for par in range(2):
                    nc.tensor.matmul(
                        st[:, par * 512 + lo:par * 512 + lo + w],
                        KT[pr][par * 64:par * 64 + 64, jlo:jlo + 128],
                        QT[pr][par * 64:par * 64 + 64,
                               ci * 512 + lo:ci * 512 + lo + w],
                        start=True, stop=True)
                ptp = pt_pool.tile([128, 1024], F16, tag="pt", name="ptp")
                if w == 512:
                    nc.scalar.activation(ptp[:, :], st[:, :], EXP, scale=SCALE)
                else:
                    st_ap = bass.AP(st.tensor, st.offset + lo,
                                    [st.ap[0], [512, 2], [1, w]])
                    pt_ap = bass.AP(ptp.tensor, ptp.offset + lo,
                                    [ptp.ap[0], [512, 2], [1, w]])
                    nc.scalar.activation(pt_ap, st_ap, EXP, scale=SCALE)
                for _ in range(FILL[ci][jb]):  # PE filler
                    if fi < len(filler):
                        filler[fi]()
                        fi += 1
                if pend is not None:  # previous slot's PV (+ pending norm)
                    pend[0]()
                    if pend[1] is not None:
                        pend[1]()
                if jb >= 4 * ci:  # causal mask on the diagonal block (gpsimd)
                    diag = bass.AP(ptp.tensor, ptp.offset + lo,
                                   [ptp.ap[0], [512, 2], [1, 128]])
                    # keep P[key x, query y] iff y - x >= 0, else 0
                    nc.gpsimd.affine_select(
                        out=diag, in_=diag,
                        compare_op=mybir.AluOpType.is_ge, fill=0.0,
                        base=0, pattern=[[0, 2], [1, 128]],
                        channel_multiplier=-1)

                def pv(pr=pr, jb=jb, njb=njb, lo=lo, w=w, ot=ot, ptp=ptp):
                    for par in range(2):
                        nc.tensor.matmul(
                            ot[:, par * 512 + lo:par * 512 + lo + w],
                            V[jb][:, pr * 192 + par * 64:pr * 192 + par * 64 + 128],
                            ptp[:, par * 512 + lo:par * 512 + lo + w],
                            start=(jb == 0), stop=(jb == njb - 1))
                last = (jb == njb - 1)
                pend = (pv, (lambda pr=pr, ci=ci, ot=ot: norm(pr, ci, ot))
                        if last else None)
        # flush at pair end
        pend[0]()
        pend[1]()
        pend = None
        while fi < len(filler):
            filler[fi]()
            fi += 1

    # ---- phase 4: output projection ----
    y_pool = ctx.enter_context(tc.tile_pool(name="ysb", bufs=3))
    for mt in range(8):
        ysb = y_pool.tile([128, C], F32, tag="y", name="ysb")
        for off, w in ((0, 512), (512, 256)):
            py = psum.tile([128, 512], F32, tag="mm", name="py", bufs=2)
            for kc in range(6):
                nc.tensor.matmul(
                    py[:, :w], OT[kc][:, mt * 128:(mt + 1) * 128],
                    WP[kc][:, off:off + w], start=(kc == 0), stop=(kc == 5))
            nc.vector.tensor_add(ysb[:, off:off + w], py[:, :w],
                                 bp[:, off:off + w])
        nc.sync.dma_start(y[mt * 128:(mt + 1) * 128, :], ysb[:])


_NC_CACHE = None


def _build():
    global _NC_CACHE
    if _NC_CACHE is not None:
        return _NC_CACHE
    nc = bacc.Bacc("TRN2", target_bir_lowering=False, debug=False,
                   num_devices=N_CORES)
    x = nc.dram_tensor("x", [T, C], F16, kind="ExternalInput").ap()
    w_attn = nc.dram_tensor("w_attn", [C, 3 * C], F16, kind="ExternalInput").ap()
    bq_d = nc.dram_tensor("bq", [C], F32, kind="ExternalInput").ap()
    bp_d = nc.dram_tensor("bp_eff", [C], F32, kind="ExternalInput").ap()
    w_proj = nc.dram_tensor("w_proj", [C, C], F16, kind="ExternalInput").ap()
    y = nc.dram_tensor("y", [T, C], F32, kind="ExternalOutput").ap()
    with tile.TileContext(nc) as tc, ExitStack() as ctx:
        _body(ctx, tc, y, x, w_attn, bq_d, bp_d, w_proj)
    nc.compile()
    _NC_CACHE = nc
    return nc


def _run(inputs, trace=False):
    nc = _build()
    x = np.asarray(inputs["x"], dtype=np.float32)
    b_attn = np.asarray(inputs["b_attn"], dtype=np.float64)
    w_proj = np.asarray(inputs["w_proj"], dtype=np.float64)
    b_proj = np.asarray(inputs["b_proj"], dtype=np.float64)
    # K bias dropped (cancels in softmax); V bias folded into output bias:
    # y = O@Wp + (bv@Wp + bp)
    bp_eff = (b_attn[2 * C:] @ w_proj + b_proj).astype(np.float32)
    shared = {
        "w_attn": np.ascontiguousarray(
            np.asarray(inputs["w_attn"], np.float32).astype(np.float16)),
        "bq": np.ascontiguousarray(b_attn[0:C].astype(np.float32)),
        "bp_eff": np.ascontiguousarray(bp_eff),
        "w_proj": np.ascontiguousarray(
            np.asarray(inputs["w_proj"], np.float32).astype(np.float16)),
    }
    x16 = x.astype(np.float16)
    in_maps = [dict(x=np.ascontiguousarray(x16[b]), **shared)
               for b in range(N_CORES)]
    res = run_bass_kernel_spmd(nc, in_maps, core_ids=list(range(N_CORES)),
                               trace=trace)
    out = np.stack([res.results[b]["y"] for b in range(N_CORES)], axis=0)
    return out.astype(np.float32), res


def kernel(**inputs):
    out, _ = _run(inputs, trace=False)
    return out


# revision 15
# speedup vs baseline: 1.2789x; 1.1366x over previous
"""Causal self-attention (B=8, T=1024, C=768, NH=12) on 8 TRN2 NeuronCores.

Sharding: pure data-parallel over batch — one batch element per core, weights
replicated. No collectives.

All matmul operands fp16 (host-cast: halves DMA, enables fast-weight-load,
1 cyc/row PE rate). K-bias dropped (cancels in softmax); V-bias folded into a
host-precomputed output bias. The program is software-pipelined so the PE
never idles: a global filler FIFO (V projection + QK projections of later
pairs) feeds 1-3 PE matmuls per attention slot, weighted toward chunk starts
to cover the PSUM-accumulator drain; PV matmuls lag one slot behind their exp.

Per-core algorithm:
  1. xT tiles via PE transposes (fp16 identity moving operand).
  2. V = x @ Wv scattered into 192-col pair-groups [V_even | ones | V_odd]:
     even-head PV lhsT [V|ones] -> PSUM rows 0:64 = O^T, 64:128 = denom;
     odd-head lhsT [ones|V] -> rows flipped.
  3. QT/KT channel-major via matmul(lhsT=Wqk col-block, rhs=xT); Q bias via
     DVE STT eviction; K bias dropped.
  4. Attention per (pair, query-chunk ci of 512, key-block jb of 128):
     ST = KT_h[jb].T @ QT_h (heads row-tiled concurrent), exp on ACT
     (scale 1/8, no max-sub), causal 0/1 mask on the diagonal block (one DVE
     mul over both heads via 3D AP), OT_aug += V_aug[jb].T @ P in PSUM.
     Normalize: 2 den copies + reciprocal + 2 PSUM-reading muls.
  5. y = OT.T @ Wp + (bv @ Wp + bp).
"""
import numpy as np
from contextlib import ExitStack

import concourse.bass as bass
import concourse.tile as tile
from concourse import bacc, mybir
from concourse.bass_utils import run_bass_kernel_spmd
from concourse.masks import make_identity, make_upper_triangular

T, C, NH, HD = 1024, 768, 12, 64
N_CORES = 8
SCALE = 1.0 / 8.0  # 1/sqrt(HD)

F32 = mybir.dt.float32
F16 = mybir.dt.float16
EXP = mybir.ActivationFunctionType.Exp


def _body(ctx, tc, y, x, w_attn, bq_d, bp_d, w_proj):
    nc = tc.nc

    const = ctx.enter_context(tc.tile_pool(name="const", bufs=1))
    persist = ctx.enter_context(tc.tile_pool(name="persist", bufs=1))
    # PSUM: st 2x[128,1024]f32 (4 banks), ot 1x[128,1024]f32 (2 banks),
    # mm 2x 1-bank (transposes / projection groups).
    psum = ctx.enter_context(tc.tile_pool(name="psum", bufs=1, space="PSUM"))

    # ---- constants ----
    ident = const.tile([128, 128], F16, tag="ident", name="ident")
    make_identity(nc, ident[:])
    # mask repeated twice so one DVE op covers both heads via a 3D AP
    mask2 = const.tile([128, 256], F16, tag="mask2", name="mask2")
    make_upper_triangular(nc, mask2[:, 0:128], val=1.0, diag=True)
    make_upper_triangular(nc, mask2[:, 128:256], val=1.0, diag=True)

    # ---- persistent tensors ----
    XT = persist.tile([128, 6 * T], F16, tag="xt", name="xt")  # kc-major
    WQK = [persist.tile([128, 2 * C], F16, tag=f"wqk{i}", name=f"wqk{i}")
           for i in range(6)]
    WV = [persist.tile([128, C], F16, tag=f"wv{i}", name=f"wv{i}")
          for i in range(6)]
    WP = [persist.tile([128, C], F16, tag=f"wp{i}", name=f"wp{i}")
          for i in range(6)]
    QT = [persist.tile([128, T], F16, tag=f"qt{i}", name=f"qt{i}") for i in range(6)]
    KT = [persist.tile([128, T], F16, tag=f"kt{i}", name=f"kt{i}") for i in range(6)]
    OT = [persist.tile([128, T], F16, tag=f"ot{i}", name=f"ot{i}") for i in range(6)]
    # V_aug: 6 pair-groups of 192 cols: [V_{2p}(64) | ones(64) | V_{2p+1}(64)]
    V = [persist.tile([128, 1152], F16, tag=f"v{i}", name=f"v{i}") for i in range(8)]
    for i in range(8):
        ones_ap = bass.AP(V[i].tensor, V[i].offset + 64,
                          [V[i].ap[0], [192, 6], [1, 64]])
        nc.gpsimd.memset(ones_ap.bitcast(F16), 1.0)

    # ---- input DMAs: x/biases on sync queue, weights on gpsimd queue.
    # WQK first (gates QK projection); WV only gates PV via the filler FIFO.
    x_pool = ctx.enter_context(tc.tile_pool(name="xs", bufs=1))
    XM = [x_pool.tile([128, C], F16, tag=f"x{mt}", name=f"xm{mt}")
          for mt in range(8)]
    for mt in range(4):
        nc.sync.dma_start(XM[mt][:], x[mt * 128:(mt + 1) * 128, :])
    for k in range(6):
        nc.gpsimd.dma_start(WQK[k][:], w_attn[k * 128:(k + 1) * 128, 0:2 * C])
    for mt in range(4, 8):
        nc.sync.dma_start(XM[mt][:], x[mt * 128:(mt + 1) * 128, :])
    bq = const.tile([128, 6], F32, tag="bq", name="bq")
    nc.sync.dma_start(bq[:, :], bq_d[:].rearrange("(n p) -> p n", p=128))
    bp_row = const.tile([1, C], F32, tag="bp_row", name="bp_row")
    nc.sync.dma_start(bp_row[:], bp_d[:].rearrange("(o f) -> o f", o=1))
    bp = const.tile([128, C], F32, tag="bp", name="bp")
    nc.gpsimd.partition_broadcast(bp[:], bp_row[:1, :])
    for k in range(6):
        nc.gpsimd.dma_start(WV[k][:], w_attn[k * 128:(k + 1) * 128, 2 * C:])
    for k in range(6):
        nc.gpsimd.dma_start(WP[k][:], w_proj[k * 128:(k + 1) * 128, :])

    # ---- PE warmup: junk transposes to lift the HAM clock gate early ----
    warm = psum.tile([128, 512], F32, tag="mm", name="warm", bufs=2)
    for i in range(32):
        nc.tensor.transpose(warm.bitcast(F16)[:, (i % 8) * 128:(i % 8) * 128 + 128],
                            ident[:], ident[:])

    # ---- transposes (x-gated; XT feeds everything) ----
    for mt in range(8):
        tp = psum.tile([128, C], F16, tag="mm", name="tp", bufs=2)
        for kc in range(6):
            nc.tensor.transpose(tp[:, kc * 128:(kc + 1) * 128],
                                XM[mt][:, kc * 128:(kc + 1) * 128], ident[:])
        dst = bass.AP(XT.tensor, XT.offset + mt * 128,
                      [XT.ap[0], [T, 6], [1, 128]])
        nc.vector.tensor_copy(dst, tp[:, :])

    def vproj_units(mt):
        for off, w in ((0, 512), (512, 256)):
            def vp(mt=mt, off=off, w=w):
                pv = psum.tile([128, 512], F32, tag="mm", name="pv", bufs=2)
                for kc in range(6):
                    nc.tensor.matmul(
                        pv[:, :w],
                        XT[:, kc * T + mt * 128:kc * T + (mt + 1) * 128],
                        WV[kc][:, off:off + w], start=(kc == 0), stop=(kc == 5))
                a = w // 128
                p0 = off // 128
                for par in range(2):
                    src_ap = bass.AP(pv.tensor, pv.offset + par * 64,
                                     [pv.ap[0], [128, a], [1, 64]])
                    dst_ap = bass.AP(V[mt].tensor,
                                     V[mt].offset + p0 * 192 + par * 128,
                                     [V[mt].ap[0], [192, a], [1, 64]])
                    nc.vector.tensor_copy(dst_ap.bitcast(F16), src_ap)
            yield vp

    def qk_group_ops(pr):
        for which in range(2):  # 0 = Q, 1 = K
            for mc in range(2):
                pq = psum.tile([128, 512], F32, tag="mm",
                               name=f"pq{pr}{which}{mc}", bufs=2)

                def mm(kc, pq=pq, which=which, pr=pr, mc=mc):
                    nc.tensor.matmul(
                        pq[:],
                        WQK[kc][:, which * C + pr * 128:which * C + (pr + 1) * 128],
                        XT[:, kc * T + mc * 512:kc * T + (mc + 1) * 512],
                        start=(kc == 0), stop=(kc == 5))
                for kc in range(6):
                    yield lambda kc=kc, mm=mm: mm(kc)

                def ev(pq=pq, which=which, pr=pr, mc=mc):
                    dst = (QT if which == 0 else KT)[pr][:, mc * 512:(mc + 1) * 512]
                    if which == 0:
                        nc.vector.scalar_tensor_tensor(
                            dst, pq[:], bq[:, pr:pr + 1], XT[:, 0:512],
                            op0=mybir.AluOpType.add, op1=mybir.AluOpType.bypass)
                    else:
                        nc.vector.tensor_copy(dst, pq[:])
                yield ev

    # ---- upfront PE work: QK proj pairs 0/1, V proj for key blocks 0-3 ----
    for pr in (0, 1):
        for op in qk_group_ops(pr):
            op()
    for mt in range(4):
        for op in vproj_units(mt):
            op()

    # ---- global filler FIFO: V proj mt 4-7, then QK proj pairs 2-5 ----
    filler = []
    for mt in range(4, 8):
        filler.extend(vproj_units(mt))
    for pr in range(2, 6):
        filler.extend(qk_group_ops(pr))
    fi = 0

    # ---- attention ----
    pt_pool = ctx.enter_context(tc.tile_pool(name="ptp", bufs=3))
    nrm_pool = ctx.enter_context(tc.tile_pool(name="nrm", bufs=2))

    def norm(pr, ci, ot):
        # par0 (cols 0:512): O rows 0:64, den rows 64:128;
        # par1 (cols 512:1024): den rows 0:64, O rows 64:128.
        den = nrm_pool.tile([64, 1024], F32, tag="den", name="den")
        nc.vector.tensor_copy(den[:, 0:512], ot[64:128, 0:512])
        nc.vector.tensor_copy(den[:, 512:1024], ot[0:64, 512:1024])
        recip = nrm_pool.tile([64, 1024], F32, tag="recip", name="recip")
        nc.vector.reciprocal_approx_fast(recip[:], den[:])
        nc.vector.tensor_mul(OT[pr][0:64, ci * 512:(ci + 1) * 512],
                             ot[0:64, 0:512], recip[0:64, 0:512])
        nc.vector.tensor_mul(OT[pr][64:128, ci * 512:(ci + 1) * 512],
                             ot[64:128, 512:1024], recip[0:64, 512:1024])

    # fillers per slot, front-loaded at chunk starts to cover norm drain
    FILL_EARLY = {0: [4, 3, 2, 1], 1: [3, 3, 2, 2, 2, 2, 1, 1]}   # 26/pair
    FILL_LATE = {0: [4, 3, 1, 0], 1: [2, 2, 1, 1, 1, 1, 0, 0]}    # 16/pair
    # hard prerequisites: all units of vproj / proj(pr) must be EMITTED
    # before anything that reads them (Tile cannot depend on future writes)
    REQ = {(0, 1): 8, (2, 0): 36, (3, 0): 64, (4, 0): 92, (5, 0): 120}
    for pr in range(6):
        FILL = FILL_EARLY if pr < 4 else FILL_LATE
        pend = None  # (pv_emitter, norm_emitter_or_None)
        for ci in range(2):
            while fi < REQ.get((pr, ci), 0):
                filler[fi]()
                fi += 1
            ot = psum.tile([128, 1024], F32, tag="ot", name=f"ot{pr}{ci}")
            njb = 4 * ci + 4
            for jb in range(njb):
                jlo = jb * 128
                lo = max(0, jlo - ci * 512)  # first valid col in this chunk
                w = 512 - lo
                st = psum.tile([128, 1024], F32, tag="st", name="st", bufs=2)
                for par in range(2):
                    nc.tensor.matmul(
                        st[:, par * 512 + lo:par * 512 + lo + w],
                        KT[pr][par * 64:par * 64 + 64, jlo:jlo + 128],
                        QT[pr][par * 64:par * 64 + 64,
                               ci * 512 + lo:ci * 512 + lo + w],
                        start=True, stop=True)
                ptp = pt_pool.tile([128, 1024], F16, tag="pt", name="ptp")
                if w == 512:
                    nc.scalar.activation(ptp[:, :], st[:, :], EXP, scale=SCALE)
                else:
                    st_ap = bass.AP(st.tensor, st.offset + lo,
                                    [st.ap[0], [512, 2], [1, w]])
                    pt_ap = bass.AP(ptp.tensor, ptp.offset + lo,
                                    [ptp.ap[0], [512, 2], [1, w]])
                    nc.scalar.activation(pt_ap, st_ap, EXP, scale=SCALE)
                for _ in range(FILL[ci][jb]):  # PE filler
                    if fi < len(filler):
                        filler[fi]()
                        fi += 1
                if pend is not None:  # previous slot's PV (+ pending norm)
                    pend[0]()
                    if pend[1] is not None:
                        pend[1]()
                if jb >= 4 * ci:  # causal mask on the diagonal block
                    diag = bass.AP(ptp.tensor, ptp.offset + lo,
                                   [ptp.ap[0], [512, 2], [1, 128]])
                    m2 = bass.AP(mask2.tensor, mask2.offset,
                                 [mask2.ap[0], [128, 2], [1, 128]])
                    nc.vector.tensor_mul(diag, diag, m2)

                def pv(pr=pr, jb=jb, njb=njb, lo=lo, w=w, ot=ot, ptp=ptp):
                    for par in range(2):
                        nc.tensor.matmul(
                            ot[:, par * 512 + lo:par * 512 + lo + w],
                            V[jb][:, pr * 192 + par * 64:pr * 192 + par * 64 + 128],
                            ptp[:, par * 512 + lo:par * 512 + lo + w],
                            start=(jb == 0), stop=(jb == njb - 1))
                last = (jb == njb - 1)
                pend = (pv, (lambda pr=pr, ci=ci, ot=ot: norm(pr, ci, ot))
                        if last else None)
        # flush at pair end
        pend[0]()
        pend[1]()
        pend = None
    while fi < len(filler):
        filler[fi]()
        fi += 1

    # ---- output projection ----
    y_pool = ctx.enter_context(tc.tile_pool(name="ysb", bufs=3))
    for mt in range(8):
        ysb = y_pool.tile([128, C], F32, tag="y", name="ysb")
        for off, w in ((0, 512), (512, 256)):
            py = psum.tile([128, 512], F32, tag="mm", name="py", bufs=2)
            for kc in range(6):
                nc.tensor.matmul(
                    py[:, :w], OT[kc][:, mt * 128:(mt + 1) * 128],
                    WP[kc][:, off:off + w], start=(kc == 0), stop=(kc == 5))
            nc.vector.tensor_add(ysb[:, off:off + w], py[:, :w],
                                 bp[:, off:off + w])
        nc.sync.dma_start(y[mt * 128:(mt + 1) * 128, :], ysb[:])


_NC_CACHE = None


def _build():
    global _NC_CACHE
    if _NC_CACHE is not None:
        return _NC_CACHE
    nc = bacc.Bacc("TRN2", target_bir_lowering=False, debug=False,
                   num_devices=N_CORES)
    x = nc.dram_tensor("x", [T, C], F16, kind="ExternalInput").ap()
    w_attn = nc.dram_tensor("w_attn", [C, 3 * C], F16, kind="ExternalInput").ap()
    bq_d = nc.dram_tensor("bq", [C], F32, kind="ExternalInput").ap()
    bp_d = nc.dram_tensor("bp_eff", [C], F32, kind="ExternalInput").ap()
    w_proj = nc.dram_tensor("w_proj", [C, C], F16, kind="ExternalInput").ap()
    y = nc.dram_tensor("y", [T, C], F32, kind="ExternalOutput").ap()
    with tile.TileContext(nc) as tc, ExitStack() as ctx:
        _body(ctx, tc, y, x, w_attn, bq_d, bp_d, w_proj)
    nc.compile()
    _NC_CACHE = nc
    return nc


def _run(inputs, trace=False):
    nc = _build()
    x = np.asarray(inputs["x"], dtype=np.float32)
    b_attn = np.asarray(inputs["b_attn"], dtype=np.float64)
    w_proj = np.asarray(inputs["w_proj"], dtype=np.float64)
    b_proj = np.asarray(inputs["b_proj"], dtype=np.float64)
    # K bias dropped (cancels in softmax); V bias folded into output bias:
    # y = O@Wp + (bv@Wp + bp)
    bp_eff = (b_attn[2 * C:] @ w_proj + b_proj).astype(np.float32)
    shared = {
        "w_attn": np.ascontiguousarray(
            np.asarray(inputs["w_attn"], np.float32).astype(np.float16)),
        "bq": np.ascontiguousarray(b_attn[0:C].astype(np.float32)),
        "bp_eff": np.ascontiguousarray(bp_eff),
        "w_proj": np.ascontiguousarray(
            np.asarray(inputs["w_proj"], np.float32).astype(np.float16)),
    }
    x16 = x.astype(np.float16)
    in_maps = [dict(x=np.ascontiguousarray(x16[b]), **shared)
               for b in range(N_CORES)]
    res = run_bass_kernel_spmd(nc, in_maps, core_ids=list(range(N_CORES)),
                               trace=trace)
    out = np.stack([res.results[b]["y"] for b in range(N_CORES)], axis=0)
    return out.astype(np.float32), res


def kernel(**inputs):
    out, _ = _run(inputs, trace=False)
    return out


# revision 21
# speedup vs baseline: 1.2851x; 1.0048x over previous
"""Causal self-attention (B=8, T=1024, C=768, NH=12) on 8 TRN2 NeuronCores.

Sharding: pure data-parallel over batch — one batch element per core, weights
replicated. No collectives.

All matmul operands fp16 (host-cast: halves DMA, enables fast-weight-load,
1 cyc/row PE rate). K-bias dropped (cancels in softmax); V-bias folded into a
host-precomputed output bias. The program is software-pipelined so the PE
never idles: a global filler FIFO (V projection + QK projections of later
pairs) feeds 1-3 PE matmuls per attention slot, weighted toward chunk starts
to cover the PSUM-accumulator drain; PV matmuls lag one slot behind their exp.

Per-core algorithm:
  1. xT tiles via PE transposes (fp16 identity moving operand).
  2. V = x @ Wv scattered into 192-col pair-groups [V_even | ones | V_odd]:
     even-head PV lhsT [V|ones] -> PSUM rows 0:64 = O^T, 64:128 = denom;
     odd-head lhsT [ones|V] -> rows flipped.
  3. QT/KT channel-major via matmul(lhsT=Wqk col-block, rhs=xT); Q bias via
     DVE STT eviction; K bias dropped.
  4. Attention per (pair, query-chunk ci of 512, key-block jb of 128):
     ST = KT_h[jb].T @ QT_h (heads row-tiled concurrent), exp on ACT
     (scale 1/8, no max-sub), causal 0/1 mask on the diagonal block (one DVE
     mul over both heads via 3D AP), OT_aug += V_aug[jb].T @ P in PSUM.
     Normalize: 2 den copies + reciprocal + 2 PSUM-reading muls.
  5. y = OT.T @ Wp + (bv @ Wp + bp).
"""
import numpy as np
from contextlib import ExitStack

import concourse.bass as bass
import concourse.tile as tile
from concourse import bacc, mybir
from concourse.bass_utils import run_bass_kernel_spmd
from concourse.masks import make_identity, make_upper_triangular

T, C, NH, HD = 1024, 768, 12, 64
N_CORES = 8
SCALE = 1.0 / 8.0  # 1/sqrt(HD)

F32 = mybir.dt.float32
F16 = mybir.dt.float16
EXP = mybir.ActivationFunctionType.Exp


def _body(ctx, tc, y, x, w_attn, bq_d, bp_d, w_proj):
    nc = tc.nc

    const = ctx.enter_context(tc.tile_pool(name="const", bufs=1))
    persist = ctx.enter_context(tc.tile_pool(name="persist", bufs=1))
    # PSUM: st 2x[128,1024]f32 (4 banks), ot 1x[128,1024]f32 (2 banks),
    # mm 2x 1-bank (transposes / projection groups).
    psum = ctx.enter_context(tc.tile_pool(name="psum", bufs=1, space="PSUM"))

    # ---- constants ----
    ident = const.tile([128, 128], F16, tag="ident", name="ident")
    make_identity(nc, ident[:])
    # additive causal mask (0 keep / -30000 drop), twice side by side so one
    # DVE add covers both heads of a pair via a 3D AP; applied to ST before
    # exp so the exp->PV chain stays DVE-free
    maskneg = const.tile([128, 256], F32, tag="maskneg", name="maskneg")
    nc.gpsimd.memset(maskneg[:], -30000.0)
    for h in range(2):
        nc.gpsimd.affine_select(
            out=maskneg[:, h * 128:(h + 1) * 128],
            in_=maskneg[:, h * 128:(h + 1) * 128],
            compare_op=mybir.AluOpType.is_gt, fill=0.0,
            base=0, pattern=[[-1, 128]], channel_multiplier=1)

    # ---- persistent tensors ----
    XT = persist.tile([128, 6 * T], F16, tag="xt", name="xt")  # kc-major
    WQK = [persist.tile([128, 2 * C], F16, tag=f"wqk{i}", name=f"wqk{i}")
           for i in range(6)]
    WV = [persist.tile([128, C], F16, tag=f"wv{i}", name=f"wv{i}")
          for i in range(6)]
    WP = [persist.tile([128, C], F16, tag=f"wp{i}", name=f"wp{i}")
          for i in range(6)]
    QT = [persist.tile([128, T], F16, tag=f"qt{i}", name=f"qt{i}") for i in range(6)]
    KT = [persist.tile([128, T], F16, tag=f"kt{i}", name=f"kt{i}") for i in range(6)]
    OT = [persist.tile([128, T], F16, tag=f"ot{i}", name=f"ot{i}") for i in range(6)]
    # V_aug: 6 pair-groups of 192 cols: [V_{2p}(64) | ones(64) | V_{2p+1}(64)]
    V = [persist.tile([128, 1152], F16, tag=f"v{i}", name=f"v{i}") for i in range(8)]
    for i in range(8):
        ones_ap = bass.AP(V[i].tensor, V[i].offset + 64,
                          [V[i].ap[0], [192, 6], [1, 64]])
        nc.gpsimd.memset(ones_ap.bitcast(F16), 1.0)

    # ---- input DMAs: x/biases on sync queue, weights on gpsimd queue.
    # WQK first (gates QK projection); WV only gates PV via the filler FIFO.
    x_pool = ctx.enter_context(tc.tile_pool(name="xs", bufs=1))
    XM = [x_pool.tile([128, C], F16, tag=f"x{mt}", name=f"xm{mt}")
          for mt in range(8)]
    for mt in range(4):
        nc.sync.dma_start(XM[mt][:], x[mt * 128:(mt + 1) * 128, :])
    for k in range(6):
        nc.gpsimd.dma_start(WQK[k][:], w_attn[k * 128:(k + 1) * 128, 0:2 * C])
    for mt in range(4, 8):
        nc.sync.dma_start(XM[mt][:], x[mt * 128:(mt + 1) * 128, :])
    bq = const.tile([128, 6], F32, tag="bq", name="bq")
    nc.sync.dma_start(bq[:, :], bq_d[:].rearrange("(n p) -> p n", p=128))
    bp_row = const.tile([1, C], F32, tag="bp_row", name="bp_row")
    nc.sync.dma_start(bp_row[:], bp_d[:].rearrange("(o f) -> o f", o=1))
    bp = const.tile([128, C], F32, tag="bp", name="bp")
    nc.gpsimd.partition_broadcast(bp[:], bp_row[:1, :])
    for k in range(6):
        nc.gpsimd.dma_start(WV[k][:], w_attn[k * 128:(k + 1) * 128, 2 * C:])
    for k in range(6):
        nc.gpsimd.dma_start(WP[k][:], w_proj[k * 128:(k + 1) * 128, :])

    # ---- PE warmup: junk matmuls to lift the HAM clock gate early
    # (transpose-mode does not count as PE-busy for the activity monitor) ----
    warm = psum.tile([128, 512], F32, tag="mm", name="warm", bufs=2)
    for i in range(24):
        nc.tensor.matmul(warm[:, 0:128], ident[:], ident[:],
                         start=True, stop=True)

    # ---- transposes (x-gated; XT feeds everything) ----
    for mt in range(8):
        tp = psum.tile([128, C], F16, tag="mm", name="tp", bufs=2)
        for kc in range(6):
            nc.tensor.transpose(tp[:, kc * 128:(kc + 1) * 128],
                                XM[mt][:, kc * 128:(kc + 1) * 128], ident[:])
        dst = bass.AP(XT.tensor, XT.offset + mt * 128,
                      [XT.ap[0], [T, 6], [1, 128]])
        nc.vector.tensor_copy(dst, tp[:, :])

    def vproj_units(mt):
        for off, w in ((0, 512), (512, 256)):
            def vp(mt=mt, off=off, w=w):
                pv = psum.tile([128, 512], F32, tag="mm", name="pv", bufs=2)
                for kc in range(6):
                    nc.tensor.matmul(
                        pv[:, :w],
                        XT[:, kc * T + mt * 128:kc * T + (mt + 1) * 128],
                        WV[kc][:, off:off + w], start=(kc == 0), stop=(kc == 5))
                a = w // 128
                p0 = off // 128
                for par in range(2):
                    src_ap = bass.AP(pv.tensor, pv.offset + par * 64,
                                     [pv.ap[0], [128, a], [1, 64]])
                    dst_ap = bass.AP(V[mt].tensor,
                                     V[mt].offset + p0 * 192 + par * 128,
                                     [V[mt].ap[0], [192, a], [1, 64]])
                    nc.vector.tensor_copy(dst_ap.bitcast(F16), src_ap)
            yield vp

    def qk_group_ops(pr):
        for which in range(2):  # 0 = Q, 1 = K
            for mc in range(2):
                pq = psum.tile([128, 512], F32, tag="mm",
                               name=f"pq{pr}{which}{mc}", bufs=2)

                def mm(kc, pq=pq, which=which, pr=pr, mc=mc):
                    nc.tensor.matmul(
                        pq[:],
                        WQK[kc][:, which * C + pr * 128:which * C + (pr + 1) * 128],
                        XT[:, kc * T + mc * 512:kc * T + (mc + 1) * 512],
                        start=(kc == 0), stop=(kc == 5))
                for kc in range(6):
                    yield lambda kc=kc, mm=mm: mm(kc)

                def ev(pq=pq, which=which, pr=pr, mc=mc):
                    dst = (QT if which == 0 else KT)[pr][:, mc * 512:(mc + 1) * 512]
                    if which == 0:
                        nc.vector.scalar_tensor_tensor(
                            dst, pq[:], bq[:, pr:pr + 1], XT[:, 0:512],
                            op0=mybir.AluOpType.add, op1=mybir.AluOpType.bypass)
                    else:
                        nc.vector.tensor_copy(dst, pq[:])
                yield ev

    # ---- upfront PE work: QK proj pairs 0/1, V proj for key blocks 0-3 ----
    for pr in (0, 1):
        for op in qk_group_ops(pr):
            op()
    for mt in range(4):
        for op in vproj_units(mt):
            op()

    # ---- output projection units (mt 0-3 feed pair 5's slots as filler) ----
    y_pool = ctx.enter_context(tc.tile_pool(name="ysb", bufs=3))

    def outproj_units(mt):
        ysb = y_pool.tile([128, C], F32, tag="y", name=f"ysb{mt}")
        for off, w in ((0, 512), (512, 256)):
            py = psum.tile([128, 512], F32, tag="mm", name=f"py{mt}{off}",
                           bufs=2)

            def mm(kc, py=py, mt=mt, off=off, w=w):
                nc.tensor.matmul(
                    py[:, :w], OT[kc][:, mt * 128:(mt + 1) * 128],
                    WP[kc][:, off:off + w], start=(kc == 0), stop=(kc == 5))
            for kc in range(6):
                yield lambda kc=kc, mm=mm: mm(kc)

            def ev(py=py, ysb=ysb, mt=mt, off=off, w=w, last=(off == 512)):
                nc.vector.tensor_add(ysb[:, off:off + w], py[:, :w],
                                     bp[:, off:off + w])
                if last:
                    nc.sync.dma_start(y[mt * 128:(mt + 1) * 128, :], ysb[:])
            yield ev

    # ---- global filler FIFO: V proj mt 4-7, QK proj pairs 2-5,
    # out-projection mt 0-3 ----
    filler = []
    for mt in range(4, 8):
        filler.extend(vproj_units(mt))
    for pr in range(2, 6):
        filler.extend(qk_group_ops(pr))
    for mt in range(4):
        filler.extend(outproj_units(mt))
    fi = 0

    # ---- attention ----
    pt_pool = ctx.enter_context(tc.tile_pool(name="ptp", bufs=6))
    nrm_pool = ctx.enter_context(tc.tile_pool(name="nrm", bufs=2))

    def norm(pr, ci, ot):
        # par0 (cols 0:512): O rows 0:64, den rows 64:128;
        # par1 (cols 512:1024): den rows 0:64, O rows 64:128.
        den = nrm_pool.tile([64, 1024], F32, tag="den", name="den")
        nc.vector.tensor_copy(den[:, 0:512], ot[64:128, 0:512])
        nc.vector.tensor_copy(den[:, 512:1024], ot[0:64, 512:1024])
        recip = nrm_pool.tile([64, 1024], F32, tag="recip", name="recip")
        nc.vector.reciprocal_approx_fast(recip[:], den[:])
        nc.vector.tensor_mul(OT[pr][0:64, ci * 512:(ci + 1) * 512],
                             ot[0:64, 0:512], recip[0:64, 0:512])
        nc.vector.tensor_mul(OT[pr][64:128, ci * 512:(ci + 1) * 512],
                             ot[64:128, 512:1024], recip[0:64, 512:1024])

    # fillers per slot, front-loaded at chunk starts to cover norm drain.
    # Out-projection units (index >= 120) must not be consumed before pair 5
    # chunk 1 slot 1 (their OT inputs are written by norms emitted up to the
    # (5,1,0) pend-flush), hence FILL[(5,0)] = 0 and FILL[(5,1)][0] = 0.
    FILLS = {
        0: {0: [4, 3, 2, 1], 1: [3, 3, 2, 2, 2, 2, 1, 1]},   # 26
        1: {0: [4, 3, 2, 1], 1: [3, 3, 2, 2, 2, 2, 1, 1]},   # 26
        2: {0: [4, 3, 2, 1], 1: [3, 3, 2, 2, 2, 2, 1, 1]},   # 26
        3: {0: [4, 3, 2, 1], 1: [3, 3, 2, 2, 2, 2, 1, 1]},   # 26
        4: {0: [4, 3, 1, 0], 1: [2, 2, 1, 1, 1, 1, 0, 0]},   # 16
        5: {0: [0, 0, 0, 0], 1: [0, 6, 6, 6, 6, 6, 6, 6]},   # 42
    }
    # hard prerequisites: all units of vproj / proj(pr) must be EMITTED
    # before anything that reads them (Tile cannot depend on future writes)
    REQ = {(0, 1): 8, (2, 0): 36, (3, 0): 64, (4, 0): 92, (5, 0): 120}
    for pr in range(6):
        FILL = FILLS[pr]
        pend = None  # (pv_emitter, norm_emitter_or_None)
        for ci in range(2):
            while fi < REQ.get((pr, ci), 0):
                filler[fi]()
                fi += 1
            ot = psum.tile([128, 1024], F32, tag="ot", name=f"ot{pr}{ci}")
            njb = 4 * ci + 4
            for jb in range(njb):
                jlo = jb * 128
                lo = max(0, jlo - ci * 512)  # first valid col in this chunk
                w = 512 - lo
                st = psum.tile([128, 1024], F32, tag="st", name="st", bufs=2)
                for par in range(2):
                    nc.tensor.matmul(
                        st[:, par * 512 + lo:par * 512 + lo + w],
                        KT[pr][par * 64:par * 64 + 64, jlo:jlo + 128],
                        QT[pr][par * 64:par * 64 + 64,
                               ci * 512 + lo:ci * 512 + lo + w],
                        start=True, stop=True)
                if jb >= 4 * ci:  # additive causal mask on the diagonal block
                    diag = bass.AP(st.tensor, st.offset + lo,
                                   [st.ap[0], [512, 2], [1, 128]])
                    m2 = bass.AP(maskneg.tensor, maskneg.offset,
                                 [maskneg.ap[0], [128, 2], [1, 128]])
                    nc.vector.tensor_add(diag, diag, m2)
                ptp = pt_pool.tile([128, 1024], F16, tag="pt", name="ptp")
                if w == 512:
                    nc.scalar.activation(ptp[:, :], st[:, :], EXP, scale=SCALE)
                else:
                    st_ap = bass.AP(st.tensor, st.offset + lo,
                                    [st.ap[0], [512, 2], [1, w]])
                    pt_ap = bass.AP(ptp.tensor, ptp.offset + lo,
                                    [ptp.ap[0], [512, 2], [1, w]])
                    nc.scalar.activation(pt_ap, st_ap, EXP, scale=SCALE)
                for _ in range(FILL[ci][jb]):  # PE filler
                    if fi < len(filler):
                        filler[fi]()
                        fi += 1
                if pend is not None:  # previous slot's PV (+ pending norm)
                    pend[0]()
                    if pend[1] is not None:
                        pend[1]()

                def pv(pr=pr, jb=jb, njb=njb, lo=lo, w=w, ot=ot, ptp=ptp):
                    for par in range(2):
                        nc.tensor.matmul(
                            ot[:, par * 512 + lo:par * 512 + lo + w],
                            V[jb][:, pr * 192 + par * 64:pr * 192 + par * 64 + 128],
                            ptp[:, par * 512 + lo:par * 512 + lo + w],
                            start=(jb == 0), stop=(jb == njb - 1))
                last = (jb == njb - 1)
                pend = (pv, (lambda pr=pr, ci=ci, ot=ot: norm(pr, ci, ot))
                        if last else None)
        # flush at pair end
        pend[0]()
        pend[1]()
        pend = None
    while fi < len(filler):
        filler[fi]()
        fi += 1

    # ---- output projection tail (mt 0-3 already ran as pair-5 filler) ----
    for mt in range(4, 8):
        for op in outproj_units(mt):
            op()


_NC_CACHE = None


def _build():
    global _NC_CACHE
    if _NC_CACHE is not None:
        return _NC_CACHE
    nc = bacc.Bacc("TRN2", target_bir_lowering=False, debug=False,
                   num_devices=N_CORES)
    x = nc.dram_tensor("x", [T, C], F16, kind="ExternalInput").ap()
    w_attn = nc.dram_tensor("w_attn", [C, 3 * C], F16, kind="ExternalInput").ap()
    bq_d = nc.dram_tensor("bq", [C], F32, kind="ExternalInput").ap()
    bp_d = nc.dram_tensor("bp_eff", [C], F32, kind="ExternalInput").ap()
    w_proj = nc.dram_tensor("w_proj", [C, C], F16, kind="ExternalInput").ap()
    y = nc.dram_tensor("y", [T, C], F32, kind="ExternalOutput").ap()
    with tile.TileContext(nc) as tc, ExitStack() as ctx:
        _body(ctx, tc, y, x, w_attn, bq_d, bp_d, w_proj)
    nc.compile()
    _NC_CACHE = nc
    return nc


def _run(inputs, trace=False):
    nc = _build()
    x = np.asarray(inputs["x"], dtype=np.float32)
    b_attn = np.asarray(inputs["b_attn"], dtype=np.float64)
    w_proj = np.asarray(inputs["w_proj"], dtype=np.float64)
    b_proj = np.asarray(inputs["b_proj"], dtype=np.float64)
    # K bias dropped (cancels in softmax); V bias folded into output bias:
    # y = O@Wp + (bv@Wp + bp)
    bp_eff = (b_attn[2 * C:] @ w_proj + b_proj).astype(np.float32)
    shared = {
        "w_attn": np.ascontiguousarray(
            np.asarray(inputs["w_attn"], np.float32).astype(np.float16)),
        "bq": np.ascontiguousarray(b_attn[0:C].astype(np.float32)),
        "bp_eff": np.ascontiguousarray(bp_eff),
        "w_proj": np.ascontiguousarray(
            np.asarray(inputs["w_proj"], np.float32).astype(np.float16)),
    }
    x16 = x.astype(np.float16)
    in_maps = [dict(x=np.ascontiguousarray(x16[b]), **shared)
               for b in range(N_CORES)]
    res = run_bass_kernel_spmd(nc, in_maps, core_ids=list(range(N_CORES)),
                               trace=trace)
    out = np.stack([res.results[b]["y"] for b in range(N_CORES)], axis=0)
    return out.astype(np.float32), res


def kernel(**inputs):
    out, _ = _run(inputs, trace=False)
    return out
